# revision 3
# baseline (speedup 1.0000x reference)
"""Trainium2 Bass kernel for causal bilinear self-attention (diagonal variant).

Computes, per (b, head):
    scores[t, s] = h[b, t] @ A[head] @ h[b, s]        (causal: s <= t)
    attn = softmax(scores, axis=-1)
    out[b, head, t, :] = attn[t, t] * h[b, t, :]
returned reshaped row-major to (B, T, H*d).

Only the softmax DIAGONAL is needed.  Per 128-row tile the causal score rows
are built in a 4-bank PSUM tile; the causal mask for the diagonal 128x128
block is added INSIDE PSUM by one extra PE matmul (ident^T @ cmask joins the
last accumulation group), so the softmax needs just:
  - one DVE reduce_max over the causal width (PSUM -> -rowmax),
  - one ACT exp with bias=-rowmax whose accumulator is the softmax
    denominator,
  - one DVE scalar_tensor_tensor (x ident, sum-accum) extracting the exp'd
    diagonal, a reciprocal, and a Pool dual-scalar multiply onto h[t, :].

Matmuls are single-pass float32r (TF32-like, ~3e-3 rel err vs the 2e-2
gate).  h^T is pretransposed and TF32-rounded on the host: no PE transposes
and no DVE rounding copies (the BIR verifier requires f32r matmul inputs to
be produced rounded).  g = h @ A is copied out of PSUM (rounding to f32r)
on the ACT engine.

Stage-2 scores use per-chunk single-bank PSUM tiles (bufs=8) so each bank
frees right after its reduce_max + exp, keeping the PE gap-free (the "chunk"
variant; "big1"/"bigc" 4-bank variants kept for A/B).  Engine busy per core
(TimelineSim): PE ~95 us (85%, bottleneck), DVE ~77, ACT ~60, Pool ~26;
span ~110.7 us vs the 314.4 us r3/r3 baseline.  Device-verified op notes:
tensor_mask_reduce crashes the device (any input space) and GPSIMD cannot
touch PSUM; both are avoided.

Sharding: 16 (b, head) pairs across 8 cores -> core c handles b = c // 4,
heads 2*(c%4) and 2*(c%4)+1.
"""

import os
import sys

try:
    import concourse.bass  # noqa: F401
except ImportError:  # pragma: no cover
    sys.path.insert(0, "/opt/trn_rl_repo")

import numpy as np

import concourse.bass as bass  # noqa: F401
import concourse.tile as tile
from concourse import bacc, bass_utils, mybir

B, T, D, H = 2, 2048, 512, 8
NCORES = 8
P = 128
NT = T // P      # 16 row tiles
ND = D // P      # 4 contraction chunks
SCH = 512        # score chunk width (one PSUM bank of fp32)
NEG = -1.0e30

f32 = mybir.dt.float32
f32r = mybir.dt.float32r
ALU = mybir.AluOpType
ACTF = mybir.ActivationFunctionType


VARIANT = os.environ.get("BK_V", "chunk")


def build_nc(variant=None):
    variant = variant or VARIANT
    assert variant in ("big1", "bigc", "chunk")
    nc = bacc.Bacc("TRN2", target_bir_lowering=False, debug=False)
    hb = nc.dram_tensor("hb", [T, D], f32, kind="ExternalInput")
    hTd = nc.dram_tensor("hTd", [D, T], f32r, kind="ExternalInput")
    A2 = nc.dram_tensor("A2", [2, D, D], f32r, kind="ExternalInput")
    identd = nc.dram_tensor("identd", [P, P], f32r, kind="ExternalInput")
    cm128d = nc.dram_tensor("cm128d", [P, P], f32r, kind="ExternalInput")
    cm256d = nc.dram_tensor("cm256d", [P, 2 * P], f32r, kind="ExternalInput")
    cmLd = nc.dram_tensor("cmLd", [P, 2 * P], f32r, kind="ExternalInput")
    out2 = nc.dram_tensor("out2", [2, T, D], f32, kind="ExternalOutput")

    hb_t = hb[:].rearrange("(n p) d -> p n d", p=P)        # [128, 16, 512]
    hT_t = hTd[:].rearrange("(c p) t -> p c t", p=P)       # [128, 4, 2048]
    A_t = A2[:].rearrange("h (c p) e -> p h c e", p=P)     # [128, 2, 4, 512]

    with tile.TileContext(nc) as tc:
        with (
            tc.tile_pool(name="const", bufs=1) as constp,
            tc.tile_pool(name="big", bufs=1) as big,
            tc.tile_pool(name="gpool", bufs=2) as gpool,
            tc.tile_pool(name="psum", bufs=2, space="PSUM") as psum,
            tc.tile_pool(name="psum1", bufs=8, space="PSUM") as psum1,
            tc.tile_pool(name="expp", bufs=2) as expp,
            tc.tile_pool(name="dump", bufs=2) as dump,
            tc.tile_pool(name="stats", bufs=4) as stats,
            tc.tile_pool(name="outp", bufs=3) as outp,
        ):
            A_sb = big.tile([P, 2, ND, D], f32r)
            hT = big.tile([P, ND, T], f32r)
            h_sb = big.tile([P, NT, D], f32)
            ident = constp.tile([P, P], f32r)
            cm128 = constp.tile([P, P], f32r)
            cm256 = constp.tile([P, 2 * P], f32r)
            cmL = constp.tile([P, 2 * P], f32r)

            # input DMAs in consumption order (single SP hw queue)
            for dc in range(ND):
                nc.sync.dma_start(out=A_sb[:, 0, dc], in_=A_t[:, 0, dc])
                nc.sync.dma_start(out=hT[:, dc, 0:SCH], in_=hT_t[:, dc, 0:SCH])
            nc.sync.dma_start(out=ident, in_=identd[:])
            nc.sync.dma_start(out=cm128, in_=cm128d[:])
            nc.sync.dma_start(out=cm256, in_=cm256d[:])
            nc.sync.dma_start(out=cmL, in_=cmLd[:])
            for tsl in range(1, ND):
                ts_ = slice(tsl * SCH, (tsl + 1) * SCH)
                nc.sync.dma_start(out=hT[:, :, ts_], in_=hT_t[:, :, ts_])
            for q in range(4):
                qs = slice(q * 4, (q + 1) * 4)
                nc.sync.dma_start(out=h_sb[:, qs], in_=hb_t[:, qs])
            nc.sync.dma_start(out=A_sb[:, 1], in_=A_t[:, 1])

            for hd in range(2):
                # ---- stage 1: gT[e, t] = sum_d A[d, e] * hT[d, t] ----
                gT = gpool.tile([P, ND, T], f32r, tag="g")
                for tsl in range(ND):
                    ts_ = slice(tsl * SCH, (tsl + 1) * SCH)
                    if variant == "chunk":
                        for ec in range(ND):
                            pe1 = psum1.tile([P, SCH], f32, tag="ps1",
                                             name=f"pg{tsl}_{ec}")
                            for dc in range(ND):
                                nc.tensor.matmul(
                                    pe1,
                                    A_sb[:, hd, dc, ec * P : (ec + 1) * P],
                                    hT[:, dc, ts_],
                                    start=(dc == 0),
                                    stop=(dc == ND - 1),
                                )
                            nc.vector.tensor_copy(gT[:, ec, ts_], pe1)
                    else:
                        pg = psum.tile([P, ND * SCH], f32, tag="ps")
                        for ec in range(ND):
                            for dc in range(ND):
                                nc.tensor.matmul(
                                    pg[:, ec * SCH : (ec + 1) * SCH],
                                    A_sb[:, hd, dc, ec * P : (ec + 1) * P],
                                    hT[:, dc, ts_],
                                    start=(dc == 0),
                                    stop=(dc == ND - 1),
                                )
                        pg_v = pg[:].rearrange("p (e t) -> p e t", e=ND)
                        nc.scalar.copy(gT[:, :, ts_], pg_v)  # f32 -> f32r

                # ---- stage 2 + softmax diag, per row tile ----
                desc = os.environ.get("BK_ORD", "asc") == "desc"
                order = (range(NT) if hd == 0 or not desc
                         else range(NT - 1, -1, -1))
                for i in order:
                    nch = i // 4 + 1
                    w = P * (i + 1)           # causal width of this row tile
                    wlast = (i % 4 + 1) * P   # causal width of last chunk
                    # f32r needs moving >= 256 for full rate; widen (extra
                    # cols land in PSUM beyond w and are never read)
                    w_mm = max(wlast, 2 * P)

                    negm = stats.tile([P, 1], f32, tag="negm")
                    ex = expp.tile([P, T], f32, tag="ex")
                    lsum = stats.tile([P, 1], f32, tag="lsum")
                    if variant == "chunk":
                        pts = [psum1.tile([P, SCH], f32, tag="ps1",
                                           name=f"pt{i}_{k}")
                               for k in range(nch)]
                    else:
                        ps = psum.tile([P, ND * SCH], f32, tag="ps")
                    m4 = (stats.tile([P, ND], f32, tag="m4", name="m4")
                          if variant != "big1" else None)
                    lp = (stats.tile([P, ND], f32, tag="lp", name="lp")
                          if variant == "chunk" else None)
                    for j in range(nch):
                        wj = SCH if j < nch - 1 else w_mm
                        wc = SCH if j < nch - 1 else wlast
                        pj = pts[j] if variant == "chunk" else None
                        dst = (pj[:, 0:wj] if variant == "chunk"
                               else ps[:, j * SCH : j * SCH + wj])
                        for ec in range(ND):
                            nc.tensor.matmul(
                                dst,
                                gT[:, ec, i * P : (i + 1) * P],
                                hT[:, ec, j * SCH : j * SCH + wj],
                                start=(ec == 0),
                                stop=(ec == ND - 1 and j < nch - 1),
                            )
                        last = j == nch - 1
                        if last:
                            # causal mask of the diagonal block, added in
                            # PSUM by the PE (closes the accumulation group)
                            if variant == "chunk":
                                mdst = (pj[:, wlast - 2 * P : wlast]
                                        if wlast > P else pj[:, 0 : 2 * P])
                            else:
                                mdst = (ps[:, w - 2 * P : w] if wlast > P
                                        else ps[:, w - P : w + P])
                            nc.tensor.matmul(
                                mdst, ident, cm256 if wlast > P else cmL,
                                start=False, stop=True,
                            )
                        else:
                            # close the group (stop flag on a 0-col.. reissue
                            # last ec matmul with stop): instead mark stop on
                            # the ec==ND-1 matmul by a dedicated group end:
                            pass
                        if variant != "big1":
                            nc.vector.reduce_max(
                                out=m4[:, j : j + 1],
                                in_=(pj[:, 0:wc] if variant == "chunk"
                                     else ps[:, j * SCH : j * SCH + wc]),
                                axis=mybir.AxisListType.X,
                            )
                    if variant == "big1":
                        nc.vector.reduce_max(
                            out=negm, in_=ps[:, :w],
                            axis=mybir.AxisListType.X, negate=True,
                        )
                    else:
                        nc.vector.reduce_max(
                            out=negm, in_=m4[:, :nch],
                            axis=mybir.AxisListType.X, negate=True,
                        )
                    if variant == "chunk":
                        for j in range(nch):
                            wc = SCH if j < nch - 1 else wlast
                            nc.scalar.activation(
                                out=ex[:, j * SCH : j * SCH + wc],
                                in_=pts[j][:, 0:wc], func=ACTF.Exp,
                                bias=negm, scale=1.0,
                                accum_out=lp[:, j : j + 1],
                            )
                        nc.vector.reduce_sum(
                            out=lsum, in_=lp[:, :nch],
                            axis=mybir.AxisListType.X,
                        )
                    else:
                        nc.scalar.activation(
                            out=ex[:, :w], in_=ps[:, :w], func=ACTF.Exp,
                            bias=negm, scale=1.0, accum_out=lsum,
                        )
                    # exp'd diagonal: multiply by identity, sum-accumulate
                    ediag = stats.tile([P, 1], f32, tag="ediag")
                    dmy = dump.tile([P, P], f32, tag="dmy")
                    nc.vector.scalar_tensor_tensor(
                        out=dmy, in0=ex[:, w - P : w], scalar=1.0,
                        in1=ident[:].bitcast(f32),
                        op0=ALU.mult, op1=ALU.mult, accum_out=ediag,
                    )
                    rl = stats.tile([P, 1], f32, tag="rl")
                    nc.vector.reciprocal(rl, lsum)
                    ot = outp.tile([P, D], f32, tag="ot")
                    eng = nc.vector if (hd == 1 and i < 4 and i % 2 == 0) \
                        else nc.gpsimd
                    eng.tensor_scalar(
                        out=ot, in0=h_sb[:, i, :], scalar1=ediag, scalar2=rl,
                        op0=ALU.mult, op1=ALU.mult,
                    )
                    nc.sync.dma_start(
                        out=out2[hd, i * P : (i + 1) * P, :], in_=ot
                    )

    nc.compile()
    return nc


_NC_CACHE = {}


def _get_nc(variant=None):
    key = variant or VARIANT
    if key not in _NC_CACHE:
        _NC_CACHE[key] = build_nc(key)
    return _NC_CACHE[key]


def _tf32_round(x):
    u = np.ascontiguousarray(x, dtype=np.float32).view(np.uint32)
    lsb = (u >> np.uint32(13)) & np.uint32(1)
    r = (u + np.uint32(0x0FFF) + lsb) & np.uint32(0xFFFFE000)
    return r.view(np.float32)


def _consts():
    ident = np.eye(P, dtype=np.float32)
    tri = np.triu(np.full((P, P), NEG, np.float32), 1)
    cm128 = tri
    cm256 = np.concatenate([np.zeros((P, P), np.float32), tri], axis=1)
    cmL = np.concatenate([tri, np.zeros((P, P), np.float32)], axis=1)
    return ident, cm128, np.ascontiguousarray(cm256), np.ascontiguousarray(cmL)


def make_in_maps(h, A):
    h = np.ascontiguousarray(h, dtype=np.float32)
    A = np.ascontiguousarray(A, dtype=np.float32)
    ident, cm128, cm256, cmL = _consts()
    in_maps = []
    for c in range(NCORES):
        b = c // 4
        h0 = 2 * (c % 4)
        in_maps.append({
            "hb": h[b],
            "hTd": _tf32_round(h[b].T),
            "A2": _tf32_round(A[h0 : h0 + 2]),
            "identd": ident,
            "cm128d": cm128,
            "cm256d": cm256,
            "cmLd": cmL,
        })
    return in_maps


def assemble(results):
    full = np.empty((B, H, T, D), dtype=np.float32)
    for c in range(NCORES):
        b = c // 4
        h0 = 2 * (c % 4)
        o = results[c]["out2"]
        full[b, h0] = o[0]
        full[b, h0 + 1] = o[1]
    return full.reshape(B, T, H * D)


def kernel(h, A):
    nc = _get_nc()
    res = bass_utils.run_bass_kernel_spmd(
        nc, make_in_maps(h, A), core_ids=list(range(NCORES))
    )
    return assemble(res.results)
